# revision 15
# baseline (speedup 1.0000x reference)
"""Trainium2 Bass kernel for Transformer-XL relative attention (nn_Attention).

Sharding: 8 cores = data-parallel over batch (2) x tensor-parallel over heads
(16 -> 4 per core).  Each core computes its 4 heads' attention for its batch,
a partial output projection, then ReduceScatter(add) over its batch quad;
each core LayerNorms its 512-row output shard.

v3 structure:
- Phase A: x^T / rel^T / weights DMA'd in per-128-row chunks spread across
  the SP and ACT HWDGE queues so the first matmuls start within ~5us.
  K+Rel projections first (B1 needs rkT complete), then Q, then V.
- Phase B1 (interleaved with B2): bd = rr_q . r_k per head, written to a
  flat DRAM buffer at row stride L+1 with a leading zero (the classic
  rel-shift shear).  bd for head h+2 is emitted between B2 blocks so its
  PSUM->SBUF copies overlap B2's matmul-heavy stretches.
- Phase B2 per (half, head): the shear read uses the DMA TRANSPOSE XBAR
  (16x128 tiles) so the shifted bd lands in SBUF already transposed
  [j, i].  Scores^T accumulate in PSUM as ac^T (operand-swapped matmul)
  + identity-matmul add of bd^T; exp() goes straight to SBUF as P^T and
  feeds the context matmul.  No PE transposes at all.
- Softmax denominator rides the context matmul (V+ = [V*mask | mask]).
- Output projection + ReduceScatter + LayerNorm run per 1024-token half,
  overlapping the collective with the other half's attention.  LayerNorm
  runs on DVE+ACT (center+variance fused via activation accum_out).
- PSUM (8 banks): big 2x[128,1024]f32 (scores/projections) + small 2x2KB
  (bd / den-broadcast / out-proj) + acc 2x[65,512]f32 (context).
"""

import numpy as np

B, L, D, NH, DH = 2, 2048, 1024, 16, 64
NHL = 4
P = 128
SCALE = 1.0 / np.sqrt(DH)
LN_EPS = 1e-5
N_CORES = 8

_CACHE = {}


def _build_program():
    import contextlib

    import concourse.bacc as bacc
    import concourse.mybir as mybir
    import concourse.tile as tile
    from concourse.masks import make_identity

    F32 = mybir.dt.float32
    F16 = mybir.dt.float16
    AF = mybir.ActivationFunctionType
    AX = mybir.AxisListType
    OP = mybir.AluOpType

    nc = bacc.Bacc("TRN2", target_bir_lowering=False, debug=False,
                   num_devices=N_CORES)

    xT = nc.declare_dram_parameter("xT", [D, L], F16, isOutput=False)
    relT = nc.declare_dram_parameter("relT", [D, L], F16, isOutput=False)
    xres = nc.declare_dram_parameter("xres", [512, D], F32, isOutput=False)
    Wq = nc.declare_dram_parameter("Wq", [D, 256], F16, isOutput=False)
    Wk = nc.declare_dram_parameter("Wk", [D, 256], F16, isOutput=False)
    Wv = nc.declare_dram_parameter("Wv", [D, 256], F16, isOutput=False)
    Wrel = nc.declare_dram_parameter("Wrel", [D, 256], F16, isOutput=False)
    Wout = nc.declare_dram_parameter("Wout", [256, D], F16, isOutput=False)
    rwb = nc.declare_dram_parameter("rwb", [256], F32, isOutput=False)
    rrb = nc.declare_dram_parameter("rrb", [256], F32, isOutput=False)
    mask01 = nc.declare_dram_parameter("mask01", [L], F32, isOutput=False)
    gamma = nc.declare_dram_parameter("gamma", [D], F32, isOutput=False)
    beta = nc.declare_dram_parameter("beta", [D], F32, isOutput=False)
    out = nc.declare_dram_parameter("out", [512, D], F32, isOutput=True)

    with tile.TileContext(nc) as tc:
        with contextlib.ExitStack() as _st:
            pers = _st.enter_context(tc.tile_pool(name="persist", bufs=1))
            dram = _st.enter_context(tc.tile_pool(name="dram", bufs=1, space="DRAM"))
            psBig = _st.enter_context(tc.tile_pool(name="psBig", bufs=2, space="PSUM"))
            psSmall = _st.enter_context(tc.tile_pool(name="psSmall", bufs=2, space="PSUM"))
            psAcc = _st.enter_context(tc.tile_pool(name="psAcc", bufs=2, space="PSUM"))

            ident = pers.tile([P, P], F16)
            make_identity(nc, ident[:])
            ones_r = pers.tile([1, 64], F16)
            nc.vector.memset(ones_r[:], 1.0)
            nbias = pers.tile([P, 1], F32)
            nc.vector.memset(nbias[:], -4.0)
            m01 = pers.tile([P, 16], F32)
            nc.sync.dma_start(m01[:], mask01.rearrange("(o p) -> p o", p=P))

            rwT = [pers.tile([P, L], F16, name=f"rwT{c}") for c in range(2)]
            rrT = [pers.tile([P, L], F16, name=f"rrT{c}") for c in range(2)]
            kT = [pers.tile([P, L], F16, name=f"kT{c}") for c in range(2)]
            rkT = [pers.tile([P, L], F16, name=f"rkT{c}") for c in range(2)]
            vp = [pers.tile([P, 16, DH + 1], F16, name=f"vp{h}") for h in range(NHL)]
            ctxT = [pers.tile([P, L], F16, name=f"ctxT{c}") for c in range(2)]

            def _copy(eng, dst, src):
                if eng is nc.scalar:
                    eng.copy(dst, src)
                else:
                    eng.tensor_copy(dst, src)

            # ---------- Phase A: projections ----------
            with tc.tile_pool(name="aslab", bufs=1) as a_p:
                xfull = a_p.tile([P, 8, L], F16, name="xfull")
                relfull = a_p.tile([P, 8, L], F16, name="relfull")
                wq_r = a_p.tile([P, 8, 256], F16)
                wk_r = a_p.tile([P, 8, 256], F16)
                wv_r = a_p.tile([P, 8, 256], F16)
                wl_r = a_p.tile([P, 8, 256], F16)
                # chunked loads spread over SP / ACT queues; K+x first
                for k in range(8):
                    e0, e1 = (nc.sync, nc.scalar) if k % 2 == 0 else \
                        (nc.scalar, nc.sync)
                    e0.dma_start(wk_r[:, k, :], Wk[128 * k:128 * k + 128, :])
                    e1.dma_start(xfull[:, k, :], xT[128 * k:128 * k + 128, :])
                for k in range(8):
                    e0, e1 = (nc.sync, nc.scalar) if k % 2 == 0 else \
                        (nc.scalar, nc.sync)
                    e0.dma_start(wl_r[:, k, :], Wrel[128 * k:128 * k + 128, :])
                    e1.dma_start(relfull[:, k, :], relT[128 * k:128 * k + 128, :])
                for k in range(8):
                    e0, e1 = (nc.sync, nc.scalar) if k % 2 == 0 else \
                        (nc.scalar, nc.sync)
                    e0.dma_start(wq_r[:, k, :], Wq[128 * k:128 * k + 128, :])
                    e1.dma_start(wv_r[:, k, :], Wv[128 * k:128 * k + 128, :])

                rwb_sb = a_p.tile([P, 2], F32)
                nc.sync.dma_start(rwb_sb[:], rwb.rearrange("(c p) -> p c", p=P))
                rrb_sb = a_p.tile([P, 2], F32)
                nc.sync.dma_start(rrb_sb[:], rrb.rearrange("(c p) -> p c", p=P))

                eng_rr = [nc.scalar, nc.vector]
                for ti, (wr, src, dsts) in enumerate(
                        ((wk_r, xfull, kT), (wl_r, relfull, rkT))):
                    for qs in range(2):
                        Q0 = 1024 * qs
                        for cc in range(2):
                            pk = psBig.tile([P, 1024], F32, tag="big")
                            for k in range(8):
                                for hh in range(2):
                                    nc.tensor.matmul(
                                        pk[:, 512 * hh:512 * hh + 512],
                                        wr[:, k, 128 * cc:128 * cc + 128],
                                        src[:, k, Q0 + 512 * hh:Q0 + 512 * hh + 512],
                                        start=(k == 0), stop=(k == 7))
                            _copy(eng_rr[(ti + cc) % 2],
                                  dsts[cc][:, Q0:Q0 + 1024], pk[:])
                for qs in range(2):
                    Q0 = 1024 * qs
                    for cc in range(2):
                        pq = psBig.tile([P, 1024], F32, tag="big")
                        for k in range(8):
                            for hh in range(2):
                                nc.tensor.matmul(
                                    pq[:, 512 * hh:512 * hh + 512],
                                    wq_r[:, k, 128 * cc:128 * cc + 128],
                                    xfull[:, k, Q0 + 512 * hh:Q0 + 512 * hh + 512],
                                    start=(k == 0), stop=(k == 7))
                        nc.vector.tensor_scalar_add(rwT[cc][:, Q0:Q0 + 1024],
                                                    pq[:], rwb_sb[:, cc:cc + 1])
                        nc.vector.tensor_scalar_add(rrT[cc][:, Q0:Q0 + 1024],
                                                    pq[:], rrb_sb[:, cc:cc + 1])
                for jo in range(16):
                    J0 = 128 * jo
                    pv = psSmall.tile([P, 256], F32, tag="small")
                    for k in range(8):
                        nc.tensor.matmul(pv[:], xfull[:, k, J0:J0 + 128],
                                         wv_r[:, k, :], start=(k == 0), stop=(k == 7))
                    for h in range(NHL):
                        nc.vector.tensor_scalar_mul(
                            vp[h][:, jo, 0:DH], pv[:, DH * h:DH * h + DH],
                            m01[:, jo:jo + 1])
                        nc.vector.tensor_copy(vp[h][:, jo, DH:DH + 1],
                                              m01[:, jo:jo + 1])

            # ---------- flat shear buffers (one per head) ----------
            pf_bufs = [dram.tile([L * (L + 1)], F16, name=f"pf{i}")
                       for i in range(NHL)]
            pf2d = [pf[0:L * (L + 1)].rearrange("(r c) -> r c", c=L + 1)
                    for pf in pf_bufs]
            # [L+1, L] view: row r starts at flat r*L (the shear read view)
            pfL = [pf[0:L * (L + 1)].rearrange("(r c) -> r c", c=L)
                   for pf in pf_bufs]

            with contextlib.ExitStack() as _stB:
                wt_p = _stB.enter_context(tc.tile_pool(name="wt", bufs=3))
                shT_p = _stB.enter_context(tc.tile_pool(name="shT", bufs=8))
                ptb_p = _stB.enter_context(tc.tile_pool(name="ptb", bufs=4))
                bc_p = _stB.enter_context(tc.tile_pool(name="bc", bufs=2))
                odd_p = _stB.enter_context(tc.tile_pool(name="oddt", bufs=2))
                den_p = _stB.enter_context(tc.tile_pool(name="den", bufs=1))
                oc_p = _stB.enter_context(tc.tile_pool(name="oc", bufs=3))
                wo_p = _stB.enter_context(tc.tile_pool(name="wo", bufs=1))
                ln_p = _stB.enter_context(tc.tile_pool(name="ln", bufs=2))
                lng_p = _stB.enter_context(tc.tile_pool(name="lng", bufs=1))

                bd_engs = [nc.vector, nc.vector, nc.scalar, nc.vector]

                def emit_B1(h):
                    cc, par = h // 2, h % 2
                    sA = slice(64 * par, 64 * par + 64)
                    for ic in range(16):
                        I0 = 128 * ic
                        wt = wt_p.tile([P, 2049], F16, tag="wt")
                        nc.vector.memset(wt[:, 0:1], 0.0)
                        for th in range(2):
                            pb2 = psBig.tile([P, 1024], F32, tag="big")
                            for tt in range(2):
                                t = 2 * th + tt
                                nc.tensor.matmul(
                                    pb2[:, 512 * tt:512 * tt + 512],
                                    rrT[cc][sA, I0:I0 + 128],
                                    rkT[cc][sA, 512 * t:512 * t + 512],
                                    start=True, stop=True)
                            _copy(bd_engs[(ic * 2 + th) % 4],
                                  wt[:, 1 + 1024 * th:1 + 1024 * th + 1024],
                                  pb2[:])
                        nc.gpsimd.dma_start(pf2d[h][I0:I0 + 128, :], wt[:])

                def emit_B2(half, h):
                    H0 = 1024 * half
                    cc, par = h // 2, h % 2
                    sA = slice(64 * par, 64 * par + 64)

                    ot = (odd_p.tile([64, 1024], F16, tag="odd", name="ot")
                          if par == 1 else None)
                    pc0 = psAcc.tile([65, 512], F32, tag="acc")
                    pc1 = psAcc.tile([65, 512], F32, tag="acc")
                    ptb_prev = None
                    for J in range(16):
                        J0 = 128 * J
                        # shear read, transposed by the DMA XBAR:
                        # shT[j-J0, i-H0] = flat[(i+1)*L + j] = shifted_bd[i, j]
                        shT = shT_p.tile([P, 1024], F16, tag="shT")
                        nc.sync.dma_start(
                            shT[:], pfL[h][H0 + 1:H0 + 1025, J0:J0 + 128],
                            transpose=True)
                        ps = psBig.tile([P, 1024], F32, tag="big")
                        for ii in range(2):
                            nc.tensor.matmul(
                                ps[:, 512 * ii:512 * ii + 512],
                                kT[cc][sA, J0:J0 + 128],
                                rwT[cc][sA, H0 + 512 * ii:H0 + 512 * ii + 512],
                                start=True, stop=False)
                            nc.tensor.matmul(
                                ps[:, 512 * ii:512 * ii + 512],
                                ident[:], shT[:, 512 * ii:512 * ii + 512],
                                start=False, stop=True)
                        if ptb_prev is not None:
                            for ii, pc in enumerate((pc0, pc1)):
                                nc.tensor.matmul(
                                    pc[:], vp[h][:, J - 1, :],
                                    ptb_prev[:, 512 * ii:512 * ii + 512],
                                    start=(J == 1), stop=False)
                        ptb = ptb_p.tile([P, 1024], F16, tag="ptb")
                        nc.scalar.activation(ptb[:], ps[:], AF.Exp, bias=nbias[:])
                        ptb_prev = ptb
                    for ii, pc in enumerate((pc0, pc1)):
                        nc.tensor.matmul(pc[:], vp[h][:, 15, :],
                                         ptb_prev[:, 512 * ii:512 * ii + 512],
                                         start=False, stop=True)

                    # softmax denominator + normalize
                    den_sb = den_p.tile([P, 1024], F32, tag="den_sb", name="den_sb")
                    den0 = den_p.tile([1, 1024], F32, tag="den0", name="den0")
                    rec0 = den_p.tile([1, 1024], F32, tag="rec0", name="rec0")
                    recr = den_p.tile([1, 1024], F16, tag="recr", name="recr")
                    scr = den_p.tile([1, 1024], F32, tag="scr", name="scr")
                    nc.vector.tensor_copy(den_sb[64:65, 0:512], pc0[64:65, :])
                    nc.vector.tensor_copy(den_sb[64:65, 512:1024], pc1[64:65, :])
                    nc.sync.dma_start(den0[0:1, :], den_sb[64:65, 0:1024])
                    nc.vector.reciprocal_approx_accurate(
                        rec0[0:1, :], den0[0:1, :], scr[0:1, :])
                    nc.vector.tensor_copy(recr[0:1, :], rec0[0:1, :])
                    for ii, pc in enumerate((pc0, pc1)):
                        i0 = H0 + 512 * ii
                        pb = psSmall.tile([64, 512], F32, tag="small")
                        nc.tensor.matmul(pb[:], ones_r[0:1, :],
                                         recr[0:1, 512 * ii:512 * ii + 512],
                                         start=True, stop=True)
                        bcf = bc_p.tile([64, 512], F32, tag="bc")
                        nc.scalar.copy(bcf[:], pb[:])
                        if par == 0:
                            nc.vector.tensor_mul(ctxT[cc][0:64, i0:i0 + 512],
                                                 pc[0:64, :], bcf[:])
                        else:
                            nc.vector.tensor_mul(ot[:, 512 * ii:512 * ii + 512],
                                                 pc[0:64, :], bcf[:])
                    if par == 1:
                        nc.sync.dma_start(ctxT[cc][64:128, H0:H0 + 1024], ot[:])

                # out-proj weights + LN params
                wo_r = [wo_p.tile([P, 2, 512], F16, name=f"wo{c}") for c in range(2)]
                for c in range(2):
                    nc.sync.dma_start(
                        wo_r[c][:], Wout[128 * c:128 * c + 128, :]
                        .rearrange("p (t n) -> p t n", t=2))
                gb = lng_p.tile([P, D], F32)
                nc.gpsimd.dma_start(gb[:], gamma.ap().rearrange(
                    "(o d) -> o d", o=1).to_broadcast((P, D)))
                bb = lng_p.tile([P, D], F32)
                nc.gpsimd.dma_start(bb[:], beta.ap().rearrange(
                    "(o d) -> o d", o=1).to_broadcast((P, D)))

                attn_d = dram.tile([L, D], F16)
                rs_d = dram.tile([512, D], F16)

                def emit_C(half):
                    for r in (2 * half, 2 * half + 1):
                        for ic4 in range(4):
                            I0 = 512 * r + 128 * ic4
                            for t in range(2):
                                po = psSmall.tile([P, 512], F32, tag="small")
                                for c in range(2):
                                    nc.tensor.matmul(po[:],
                                                     ctxT[c][:, I0:I0 + 128],
                                                     wo_r[c][:, t, :],
                                                     start=(c == 0), stop=(c == 1))
                                ao = oc_p.tile([P, 512], F16, tag="ao")
                                if t == 0:
                                    nc.vector.tensor_copy(ao[:], po[:])
                                else:
                                    nc.scalar.copy(ao[:], po[:])
                                nc.sync.dma_start(
                                    attn_d[I0:I0 + 128, 512 * t:512 * t + 512],
                                    ao[:])
                        nc.gpsimd.collective_compute(
                            "ReduceScatter", OP.add,
                            replica_groups=[[0, 1, 2, 3], [4, 5, 6, 7]],
                            ins=[attn_d[512 * r:512 * r + 512, :].opt()],
                            outs=[rs_d[128 * r:128 * r + 128, :].opt()],
                        )
                    for r in (2 * half, 2 * half + 1):
                        R0 = 128 * r
                        zt16 = ln_p.tile([P, D], F16, tag="zt16")
                        nc.gpsimd.dma_start(zt16[:], rs_d[R0:R0 + 128, :])
                        zt = ln_p.tile([P, D], F32, tag="zt")
                        nc.scalar.copy(zt[:], zt16[:])
                        xr = ln_p.tile([P, D], F32, tag="xr")
                        nc.gpsimd.dma_start(xr[:], xres[R0:R0 + 128, :])
                        nc.vector.tensor_add(zt[:], zt[:], xr[:])
                        negmu = ln_p.tile([P, 1], F32, tag="negmu")
                        nc.vector.tensor_reduce(negmu[:], zt[:], AX.X, OP.add)
                        nc.vector.tensor_scalar_mul(negmu[:], negmu[:], -1.0 / D)
                        nc.vector.tensor_scalar_add(zt[:], zt[:], negmu[:])
                        var = ln_p.tile([P, 1], F32, tag="var")
                        nc.scalar.activation(zt16[:], zt[:], AF.Square,
                                             accum_out=var[:])
                        nc.vector.tensor_scalar_mul(var[:], var[:], 1.0 / D)
                        nc.vector.tensor_scalar_add(var[:], var[:], LN_EPS)
                        sd = ln_p.tile([P, 1], F32, tag="sd")
                        nc.scalar.activation(sd[:], var[:], AF.Sqrt)
                        isd = ln_p.tile([P, 1], F32, tag="isd")
                        nc.vector.reciprocal(isd[:], sd[:])
                        nc.scalar.activation(xr[:], zt[:], AF.Copy, scale=isd[:])
                        nc.vector.tensor_mul(xr[:], xr[:], gb[:])
                        nc.vector.tensor_add(xr[:], xr[:], bb[:])
                        nc.sync.dma_start(out[R0:R0 + 128, :], xr[:])

                # ---------- emission schedule ----------
                emit_B1(0)
                emit_B1(1)
                emit_B2(0, 0)
                emit_B1(2)
                emit_B2(0, 1)
                emit_B1(3)
                emit_B2(0, 2)
                emit_B2(0, 3)
                emit_C(0)
                for h in range(NHL):
                    emit_B2(1, h)
                emit_C(1)

    nc.compile()
    return nc


def _prep_inputs(x, relative_pos, r_w_bias, r_r_bias, attn_mask,
                 W_qkv, W_rel, W_out, ln_gamma, ln_beta):
    in_maps = []
    relT = np.ascontiguousarray(relative_pos.T).astype(np.float16)
    m01f = (~np.asarray(attn_mask).astype(bool)).astype(np.float32)
    for c in range(N_CORES):
        b, g = c // 4, c % 4
        h0 = 4 * g
        cols = slice(DH * h0, DH * h0 + 256)
        im = dict(
            xT=np.ascontiguousarray(x[b].T).astype(np.float16),
            relT=relT,
            xres=np.ascontiguousarray(np.concatenate(
                [x[b, 512 * r + 128 * g: 512 * r + 128 * g + 128, :]
                 for r in range(4)], axis=0)).astype(np.float32),
            Wq=np.ascontiguousarray(
                W_qkv[:, DH * h0:DH * h0 + 256] * SCALE).astype(np.float16),
            Wk=np.ascontiguousarray(
                W_qkv[:, D + DH * h0: D + DH * h0 + 256]).astype(np.float16),
            Wv=np.ascontiguousarray(
                W_qkv[:, 2 * D + DH * h0: 2 * D + DH * h0 + 256]).astype(np.float16),
            Wrel=np.ascontiguousarray(W_rel[:, cols]).astype(np.float16),
            Wout=np.ascontiguousarray(W_out[cols, :]).astype(np.float16),
            rwb=np.ascontiguousarray(
                r_w_bias[h0:h0 + 4].reshape(-1) * SCALE).astype(np.float32),
            rrb=np.ascontiguousarray(
                r_r_bias[h0:h0 + 4].reshape(-1) * SCALE).astype(np.float32),
            mask01=m01f[b],
            gamma=np.asarray(ln_gamma).astype(np.float32),
            beta=np.asarray(ln_beta).astype(np.float32),
        )
        in_maps.append(im)
    return in_maps


def kernel(**inputs):
    from concourse.bass_utils import run_bass_kernel_spmd

    if "nc" not in _CACHE:
        _CACHE["nc"] = _build_program()
    nc = _CACHE["nc"]

    in_maps = _prep_inputs(**{k: np.asarray(v) for k, v in inputs.items()})
    res = run_bass_kernel_spmd(nc, in_maps, list(range(N_CORES)))
    outp = np.empty((B, L, D), np.float32)
    for c in range(N_CORES):
        b, g = c // 4, c % 4
        o = res.results[c]["out"]
        for r in range(4):
            outp[b, 512 * r + 128 * g: 512 * r + 128 * g + 128, :] = \
                o[128 * r:128 * r + 128, :]
    return outp
